# revision 61
# baseline (speedup 1.0000x reference)
"""nn_LocalSpatialEncoding Trainium2 kernel (Bass/Tile, 8 NeuronCores).

Takes the FULL inputs of the reference problem (B=4, N=16384, K=16, D=16),
shards over (batch, point-range) across 8 cores, runs one SPMD Bass kernel,
and reassembles the full output.

Decomposition of the conv (channel c, point n, neighbor k):
    x[c,n,k] = A'[c]@[coords[n],1] + Q[c, idx[n,k]] + e[c]*dist[n,k]
    A' = w[:,0:3]+w[:,6:9] (+bias col), Q = (w[:,3:6]-w[:,6:9])@coords^T,
    e = w[:,9]
Device-side work per core:
  - gpsimd ap_gather of the host-precomputed 16-channel table Q
    (replicated per 16-partition slab) -> qg
  - PE matmul of a host-packed rhs [coords-broadcast | dist] (bf16) with a
    block-diagonal lhsT -> psum = A'-term + e*dist
  - DVE: X = qg + psum (bf16 out, accum -> per-channel sum)
  - DVE/Act: X^2 accum -> per-channel sumsq
  - AllReduce of (16,2) channel stats, scale/bias compute
  - Act: relu(scale*X+bias) -> bf16 out; feats half is a DRAM->DRAM copy
    of host-broadcast features.
Output tensor is bf16 on device; host converts to float32.
"""
import numpy as np
from contextlib import ExitStack

import ml_dtypes
import concourse.bacc as bacc
import concourse.tile as tile
from concourse import mybir
from concourse.bass_utils import run_bass_kernel_spmd

F32 = mybir.dt.float32
BF16 = mybir.dt.bfloat16
I16 = mybir.dt.int16
NPBF = ml_dtypes.bfloat16
EPS = 1e-6
K = 16
D = 16
NSLAB = 8

# full-problem config (hardcoded)
B = 4
N = 16384
NL = 8192            # points per core
N_CORES = 8
Mslab = NL * K // NSLAB   # 16384 columns of X per 16-partition slab
PL = NL // NSLAB          # 1024 points per slab
COUNT = B * N * K

CH = 2048            # columns per DVE/Act phase-1 chunk
NCH = Mslab // CH    # 8
MM = 512             # columns per matmul (one PSUM bank)
NG = 4               # number of ap_gather calls
GCH = Mslab // NG    # 4096 idxs per gather
CHO = 16384          # columns per finalize relu op
NWR = 4              # output-write DMA splits per relu op
# square-pass segments: (start col, width, engine). Split balances measured
# HW rates: scalar 0.869 ns/col vs vector 1.115 ns/col plus fixed blocks.
SQ_SEGS = [(0, 8192, 'act'), (8192, 3072, 'act'),
           (11264, 2560, 'dve'), (13824, 2560, 'dve')]
NSQ_ACT = sum(1 for s in SQ_SEGS if s[2] == 'act')
NSQ_DVE = sum(1 for s in SQ_SEGS if s[2] == 'dve')

IN_NAMES = ['qrep', 'idxw', 'rall', 'featfull', 'lhsT_R',
            'gb16', 'red16', 'rep128']
SHAPES = dict(qrep=(128, N), idxw=(128, Mslab // 16), rall=(40, Mslab),
              featfull=(128, Mslab), lhsT_R=(40, 128), gb16=(16, 2),
              red16=(128, 16), rep128=(16, 128))
DTYPES = dict(idxw=I16, rall=BF16, featfull=BF16, lhsT_R=BF16)
OUT_SHAPE = (2 * D, NL, K)
OUT_DT = BF16


def _prep_params(conv_w, conv_b, gamma, beta):
    A = np.concatenate(
        [conv_w[:, 0:3] + conv_w[:, 6:9], conv_b[:, None]], axis=1
    ).astype(np.float32)                      # (16, 4)
    C = (conv_w[:, 3:6] - conv_w[:, 6:9]).astype(np.float32)  # (16, 3)
    e = conv_w[:, 9].astype(np.float32)       # (16,)

    lhsT_R = np.zeros((40, 128), np.float32)
    for a in range(NSLAB):
        lhsT_R[4 * a:4 * a + 4, 16 * a:16 * a + 16] = A.T
        lhsT_R[32 + a, 16 * a:16 * a + 16] = e
    gb16 = np.stack([gamma, beta], axis=1).astype(np.float32)
    # red16 folds the 1/COUNT mean normalization into the reduction
    red16 = np.zeros((128, 16), np.float32)
    rep128 = np.zeros((16, 128), np.float32)
    eye = np.eye(16, dtype=np.float32)
    for a in range(NSLAB):
        red16[16 * a:16 * a + 16, :] = eye / COUNT
        rep128[:, 16 * a:16 * a + 16] = eye
    return dict(C=C, lhsT_R=lhsT_R.astype(NPBF), gb16=gb16,
                red16=red16, rep128=rep128)


def _prep_core(coords_b, idx_s, dist_s, feat_s, params, n0):
    # gather table: Q = C @ coords^T, replicated across the 8 slabs
    Q = params['C'] @ coords_b.T                        # (16, N) f32
    qrep = np.ascontiguousarray(np.tile(Q, (NSLAB, 1)).astype(np.float32))

    # idx packing: row 16a+p holds slab a's indices at positions = p mod 16
    idx_flat = idx_s.reshape(NSLAB, Mslab)
    idxw = np.zeros((128, Mslab // 16), np.int16)
    for p in range(16):
        idxw[p::16, :] = idx_flat[:, p::16]

    # rall: rows 0..31 = [coords,1] broadcast along k; rows 32..39 = dist
    rall = np.zeros((40, Mslab), np.float32)
    cs = coords_b[n0:n0 + NL].reshape(NSLAB, PL, 3)     # (8, 1024, 3)
    for a in range(NSLAB):
        for j in range(3):
            rall[4 * a + j] = np.repeat(cs[a, :, j], K)
        rall[4 * a + 3] = 1.0
        rall[32 + a] = dist_s[a * PL:(a + 1) * PL].reshape(-1)
    rall = rall.astype(NPBF)

    # feats half, pre-broadcast to the output layout: row c*8+a, col (m k)
    featfull = np.broadcast_to(
        feat_s.reshape(16, NSLAB, PL, 1), (16, NSLAB, PL, K)
    ).reshape(128, Mslab).astype(NPBF)

    d = dict(qrep=qrep, idxw=idxw, rall=rall, featfull=featfull)
    for k in ('lhsT_R', 'gb16', 'red16', 'rep128'):
        d[k] = params[k]
    return d


def shard_inputs(coords, features, idx, dist, conv_w, conv_b, gamma, beta):
    params = _prep_params(conv_w, conv_b, gamma, beta)
    per_core = []
    for c in range(N_CORES):
        b, h = c // 2, c % 2
        sl = slice(h * NL, (h + 1) * NL)
        per_core.append(_prep_core(
            np.asarray(coords[b]), np.asarray(idx[b][sl]),
            np.asarray(dist[b][sl]), np.asarray(features[b, :, sl, 0]),
            params, h * NL))
    return per_core


def build_kernel(tc, outs, ins, use_collective=True, repeat=1):
    for _r in range(repeat):
        _build_once(tc, outs, ins, use_collective, f"r{_r}" if repeat > 1
                    else "")


def _build_once(tc, outs, ins, use_collective, pfx):
    nc = tc.nc
    t = dict(zip(IN_NAMES, ins))
    out_d = outs[0]

    ctx = ExitStack()
    sb = ctx.enter_context(tc.tile_pool(name=pfx + "fixed", bufs=1))
    ps = ctx.enter_context(tc.tile_pool(name=pfx + "psum", bufs=2,
                                        space="PSUM"))
    dram = ctx.enter_context(tc.tile_pool(name=pfx + "dram", bufs=1,
                                          space="DRAM"))
    ld_ctx = ExitStack()
    ld = ld_ctx.enter_context(tc.tile_pool(name=pfx + "qtab", bufs=1))

    # ---------- preamble loads (big tables via gpsimd SWDGE queue) ----------
    qrep_t = ld.tile([128, N], F32)
    for u in range(4):
        nc.gpsimd.dma_start(out=qrep_t[:, u * 4096:(u + 1) * 4096],
                            in_=t['qrep'][:][:, u * 4096:(u + 1) * 4096])
    idx_t = sb.tile([128, Mslab // 16], I16)
    nc.sync.dma_start(out=idx_t[:], in_=t['idxw'][:])
    lhsT_R_t = sb.tile([40, 128], BF16)
    nc.sync.dma_start(out=lhsT_R_t[:], in_=t['lhsT_R'][:])
    gb_t = sb.tile([16, 2], F32)
    nc.sync.dma_start(out=gb_t[:], in_=t['gb16'][:])
    red_t = sb.tile([128, 16], F32)
    nc.sync.dma_start(out=red_t[:], in_=t['red16'][:])
    rep_t = sb.tile([16, 128], F32)
    nc.sync.dma_start(out=rep_t[:], in_=t['rep128'][:])
    eps_t = sb.tile([16, 1], F32)
    nc.vector.memset(eps_t[:], EPS)
    # warm the act table with a set containing Sqrt (which also holds
    # Square/Relu/Copy) so the whole kernel needs only one table load
    warm_t = sb.tile([16, 1], F32)
    nc.scalar.activation(out=warm_t[:], in_=eps_t[:],
                         func=mybir.ActivationFunctionType.Sqrt)

    x_view = out_d[:][0:16, :, :].rearrange("c (a m) k -> a c (m k)", a=NSLAB)

    # ---------- feats half: DRAM->DRAM copy of host-broadcast features ----
    fv = out_d[:][16:32, :, :].rearrange("c (a m) k -> (c a) (m k)", a=NSLAB)
    for u in range(4):
        nc.gpsimd.dma_start(out=fv[:, u * 4096:(u + 1) * 4096],
                            in_=t['featfull'][:][:, u * 4096:(u + 1) * 4096])

    # ---------- phase 1: gather + matmul + combine + stats ----------
    qg = [sb.tile([128, GCH], F32, name=f"{pfx}qg{g}") for g in range(NG)]
    for g in range(NG):
        nc.gpsimd.ap_gather(
            out_ap=qg[g][:].unsqueeze(2), in_ap=qrep_t[:].unsqueeze(2),
            idxs_ap=idx_t[:, g * (GCH // 16):(g + 1) * (GCH // 16)],
            channels=128, num_elems=N, d=1, num_idxs=GCH)

    Xb = sb.tile([128, Mslab], BF16)
    s1col = sb.tile([128, NCH], F32)
    s2a = sb.tile([128, NSQ_ACT], F32)
    s2d = sb.tile([128, NSQ_DVE], F32)
    sq_by_end = {}
    for (sc0, w, eng) in SQ_SEGS:
        jend = (sc0 + w + CH - 1) // CH - 1
        sq_by_end[jend] = (sc0, w, eng)

    rall_t = []
    for u in range(4):
        rt = sb.tile([40, 4096], BF16, tag="rl", bufs=2,
                     name=f"{pfx}rall{u}")
        rall_t.append(rt)
    nc.sync.dma_start(out=rall_t[0][:], in_=t['rall'][:][:, 0:4096])
    nc.sync.dma_start(out=rall_t[1][:], in_=t['rall'][:][:, 4096:8192])

    na = nd = 0
    for j in range(NCH):
        c0 = j * CH
        if c0 % 4096 == 0 and c0 // 4096 + 2 < 4:
            u = c0 // 4096 + 2
            nc.sync.dma_start(out=rall_t[u][:],
                              in_=t['rall'][:][:, u * 4096:(u + 1) * 4096])
        pt = ps.tile([128, CH], F32, tag="pp", bufs=2, name=f"{pfx}pp{j}")
        rt = rall_t[c0 // 4096]
        r0 = c0 % 4096
        for h in range(CH // MM):
            nc.tensor.matmul(out=pt[:, h * MM:(h + 1) * MM], lhsT=lhsT_R_t[:],
                             rhs=rt[:, r0 + h * MM:r0 + (h + 1) * MM],
                             start=True, stop=True)
        g = c0 // GCH
        q0 = c0 % GCH
        nc.vector.scalar_tensor_tensor(
            out=Xb[:, c0:c0 + CH], in0=qg[g][:, q0:q0 + CH], scalar=1.0,
            in1=pt[:], op0=mybir.AluOpType.mult, op1=mybir.AluOpType.add,
            accum_out=s1col[:, j:j + 1])

        if j in sq_by_end:
            sc0, w, eng = sq_by_end[j]
            sq = sb.tile([128, w], BF16, tag=f"sq{eng}", bufs=1,
                         name=f"{pfx}sq{j}")
            if eng == 'dve':
                nc.vector.scalar_tensor_tensor(
                    out=sq[:], in0=Xb[:, sc0:sc0 + w], scalar=1.0,
                    in1=Xb[:, sc0:sc0 + w], op0=mybir.AluOpType.mult,
                    op1=mybir.AluOpType.mult, accum_out=s2d[:, nd:nd + 1])
                nd += 1
            else:
                nc.scalar.activation(
                    out=sq[:], in_=Xb[:, sc0:sc0 + w],
                    func=mybir.ActivationFunctionType.Square,
                    accum_out=s2a[:, na:na + 1])
                na += 1

    # ---------- stats: reduce, all-reduce, scale/bias ----------
    stats2 = sb.tile([128, 2], F32)
    nc.vector.tensor_reduce(out=stats2[:, 0:1], in_=s1col[:],
                            axis=mybir.AxisListType.X, op=mybir.AluOpType.add)
    s2sum = sb.tile([128, 2], F32)
    nc.vector.tensor_reduce(out=s2sum[:, 0:1], in_=s2a[:],
                            axis=mybir.AxisListType.X, op=mybir.AluOpType.add)
    nc.vector.tensor_reduce(out=s2sum[:, 1:2], in_=s2d[:],
                            axis=mybir.AxisListType.X, op=mybir.AluOpType.add)
    nc.vector.tensor_tensor(out=stats2[:, 1:2], in0=s2sum[:, 0:1],
                            in1=s2sum[:, 1:2], op=mybir.AluOpType.add)
    ps16 = ps.tile([128, CH], F32, tag="pp", name=pfx + "ps16")
    nc.tensor.matmul(out=ps16[0:16, 0:2], lhsT=red_t[:], rhs=stats2[:],
                     start=True, stop=True)
    sb16 = sb.tile([16, 2], F32)
    nc.vector.tensor_copy(out=sb16[:], in_=ps16[0:16, 0:2])

    cc_in = dram.tile([16, 2], F32)
    cc_out = dram.tile([16, 2], F32)
    nc.gpsimd.dma_start(out=cc_in[:], in_=sb16[:])
    if use_collective:
        nc.gpsimd.collective_compute(
            "AllReduce", mybir.AluOpType.add,
            replica_groups=[list(range(N_CORES))],
            ins=[cc_in.opt()], outs=[cc_out.opt()])
    else:
        nc.gpsimd.dma_start(out=cc_out[:], in_=cc_in[:])
    g16 = sb.tile([16, 2], F32)
    nc.gpsimd.dma_start(out=g16[:], in_=cc_out[:])

    var16 = sb.tile([16, 1], F32)
    nc.vector.tensor_tensor(out=var16[:], in0=g16[:, 0:1], in1=g16[:, 0:1],
                            op=mybir.AluOpType.mult)
    nc.vector.tensor_tensor(out=var16[:], in0=g16[:, 1:2], in1=var16[:],
                            op=mybir.AluOpType.subtract)
    std16 = sb.tile([16, 1], F32)
    nc.scalar.activation(out=std16[:], in_=var16[:],
                         func=mybir.ActivationFunctionType.Sqrt,
                         bias=eps_t[:, 0:1])
    rstd16 = sb.tile([16, 1], F32)
    nc.vector.reciprocal(out=rstd16[:], in_=std16[:])
    sc16 = sb.tile([16, 2], F32)
    nc.vector.tensor_tensor(out=sc16[:, 0:1], in0=gb_t[:, 0:1], in1=rstd16[:],
                            op=mybir.AluOpType.mult)
    tmu = sb.tile([16, 1], F32)
    nc.vector.tensor_tensor(out=tmu[:], in0=g16[:, 0:1], in1=sc16[:, 0:1],
                            op=mybir.AluOpType.mult)
    nc.vector.tensor_tensor(out=sc16[:, 1:2], in0=gb_t[:, 1:2], in1=tmu[:],
                            op=mybir.AluOpType.subtract)
    psr = ps.tile([128, CH], F32, tag="pp", name=pfx + "psr")
    nc.tensor.matmul(out=psr[:, 0:2], lhsT=rep_t[:], rhs=sc16[:],
                     start=True, stop=True)
    sb_col = sb.tile([128, 2], F32)
    nc.vector.tensor_copy(out=sb_col[:], in_=psr[:, 0:2])

    ld_ctx.close()
    st = ctx.enter_context(tc.tile_pool(name=pfx + "stream", bufs=2))

    # ---------- finalize: relu(x*s0+s1) -> DRAM (bf16) ----------
    ox = st.tile([128, CHO], BF16, tag="ox", name=f"{pfx}ox")
    nc.scalar.activation(
        out=ox[:], in_=Xb[:],
        func=mybir.ActivationFunctionType.Relu,
        scale=sb_col[:, 0:1], bias=sb_col[:, 1:2])
    w = CHO // NWR
    for u in range(NWR):
        nc.sync.dma_start(out=x_view[:, :, u * w:(u + 1) * w],
                          in_=ox[:, u * w:(u + 1) * w])

    ctx.close()


_COMPILED = None


def _get_compiled():
    global _COMPILED
    if _COMPILED is not None:
        return _COMPILED
    nc = bacc.Bacc("TRN2", target_bir_lowering=False, debug=False,
                   num_devices=N_CORES)
    in_aps = []
    for name in IN_NAMES:
        in_aps.append(nc.dram_tensor(
            name, SHAPES[name], DTYPES.get(name, F32),
            kind="ExternalInput").ap())
    out_ap = nc.dram_tensor("out", OUT_SHAPE, OUT_DT,
                            kind="ExternalOutput").ap()
    with tile.TileContext(nc) as tc:
        build_kernel(tc, [out_ap], in_aps)
    nc.compile()
    _COMPILED = nc
    return nc


def run_sharded(per_core, trace=False, **kw):
    nc = _get_compiled()
    in_maps = [{k: pc[k] for k in IN_NAMES} for pc in per_core]
    return run_bass_kernel_spmd(nc, in_maps, list(range(N_CORES)),
                                trace=trace, **kw)


def unshard_core(arr):
    return np.asarray(arr).astype(np.float32)


def kernel(coords, features, idx, dist, conv_w, conv_b, bn_gamma, bn_beta):
    coords = np.asarray(coords, dtype=np.float32)
    features = np.asarray(features, dtype=np.float32)
    idx = np.asarray(idx)
    dist = np.asarray(dist, dtype=np.float32)
    conv_w = np.asarray(conv_w, dtype=np.float32)
    conv_b = np.asarray(conv_b, dtype=np.float32)
    bn_gamma = np.asarray(bn_gamma, dtype=np.float32)
    bn_beta = np.asarray(bn_beta, dtype=np.float32)

    per_core = shard_inputs(coords, features, idx, dist, conv_w, conv_b,
                            bn_gamma, bn_beta)
    res = run_sharded(per_core)
    out = np.empty((B, 2 * D, N, K), np.float32)
    for c in range(N_CORES):
        b, h = c // 2, c % 2
        out[b, :, h * NL:(h + 1) * NL, :] = unshard_core(res.results[c]['out'])
    return out
